# revision 1
# baseline (speedup 1.0000x reference)
"""Trainium2 Bass kernel for the NeuralBloch ODE problem.

Math: dense-output neural ODE solved by Hermite-Simpson collocation
(order-4) with windowed Picard fixed-point iteration.  Each core owns a
256-row batch shard; time is processed in windows of TW=32 grid
intervals; within a window, K=2 sweeps each evaluate the MLP vector
field at all window grid points and interval midpoints as large batched
matmuls (features on partitions, (time x batch) samples on the free
dim), then a short serial trapezoid-like scan updates the window's y
values in place.

Per sweep m (uniform step h, window intervals j):
  f_j   = MLP(y^m_j, u_j, p, t_j)
  ymid_j = (y^m_j + y^m_{j+1})/2 + (h/8)(f_j - f_{j+1})
  fm_j  = MLP(ymid_j, umid_j, p, tmid_j)
  y^{m+1}_{j+1} = y^{m+1}_j + (h/6) f_j + (2h/3) fm_j + (h/6) f_{j+1}

On-chip trick: ymid never materializes.  Its contribution to the mid
MLP's first-layer preactivation is accumulated directly in PSUM from
  0.5*W1y^T y_j + 0.5*W1y^T y_{j+1} + M8^T h2_j + (-M8)^T h2_{j+1}
where M8 = (h/8) * W3 @ W1y uses the grid evals' second-layer
activations h2 (the b3 terms cancel).  Scales h/6 and 2h/3 are folded
into separate copies of W3/b3 for the grid and mid output layers.
"""

import numpy as np

B_FULL = 2048
T_FULL = 2048
HID = 128
NCORES = 8
BC = B_FULL // NCORES  # 256
TW = 28                # grid intervals per window
K_SWEEPS = 2
F32 = np.float32

_CACHE = {}


def _windows(T, tw):
    out = []
    i0 = 0
    while i0 < T - 1:
        J = min(tw, T - 1 - i0)
        out.append((i0, J))
        i0 += J
    return out


def _pieces(n, k=None, step=8):
    """Split range(n) into contiguous [a,b) pieces of `step` slices
    (4-superchunk aligned so each mm reads from exactly one piece)."""
    out = []
    a = 0
    while a < n:
        b = min(n, a + step)
        out.append((a, b))
        a = b
    return out


def _superchunks(nslices, max_slices=4):
    """Split `nslices` 256-col slices into chunks of <=4 slices."""
    out = []
    s0 = 0
    while s0 < nslices:
        w = min(max_slices, nslices - s0)
        out.append((s0, w))
        s0 += w
    return out


def _build_nc(T, tw, ksweeps, skip=(), cfg=None):
    import concourse.bass as bass
    import concourse.bacc as bacc
    import concourse.mybir as mybir
    from concourse.tile import TileContext

    skip = set(skip)
    cfg = dict(cfg or {})
    bufs_ps = cfg.get("ps", 3)
    bufs_psC = cfg.get("psC", 2)
    bufs_h1 = cfg.get("h1", 3)
    fp32mm = cfg.get("fp32mm", False)
    bufs_h2m = cfg.get("h2m", 2)
    gsbuf = cfg.get("gsbuf", False)

    f32 = mybir.dt.float32
    f32r = mybir.dt.float32r
    Tanh = mybir.ActivationFunctionType.Tanh
    ADD = mybir.AluOpType.add

    def mm(out, lhsT, rhs, start, stop):
        if fp32mm:
            lhsT = lhsT.bitcast(f32)
            rhs = rhs.bitcast(f32)
        nc.tensor.matmul(out, lhsT, rhs, start=start, stop=stop)

    SMAX = tw + 1
    nc = bacc.Bacc(None)
    consts_d = nc.dram_tensor("consts", [128, 1536], f32r, kind="ExternalInput")
    bias_d = nc.dram_tensor("biases", [128, 8], f32, kind="ExternalInput")
    ugt_d = nc.dram_tensor("ugt", [T, 5, BC], f32r, kind="ExternalInput")
    umt_d = nc.dram_tensor("umt", [T - 1, 5, BC], f32r, kind="ExternalInput")
    y0_d = nc.dram_tensor("y0t", [3, BC], f32, kind="ExternalInput")
    out_d = nc.dram_tensor("out", [T, 3, BC], f32, kind="ExternalOutput")

    wins = _windows(T, tw)

    with TileContext(nc) as tc:
        with (
            tc.tile_pool(name="const", bufs=1) as cpool,
            tc.tile_pool(name="big", bufs=1) as bigpool,
            tc.tile_pool(name="h1", bufs=bufs_h1) as h1pool,
            tc.tile_pool(name="h2m", bufs=bufs_h2m) as h2mpool,
            tc.tile_pool(name="gs", bufs=4) as gspool,
            tc.tile_pool(name="ys", bufs=2) as yspool,
            tc.tile_pool(name="ps", bufs=bufs_ps, space="PSUM") as pspool,
            tc.tile_pool(name="psC", bufs=bufs_psC, space="PSUM") as psCpool,
        ):
            C0 = cpool.tile([128, 1536], f32r)
            nc.gpsimd.dma_start(C0[:, :], consts_d[:, :])
            Cb = cpool.tile([128, 8], f32)
            nc.gpsimd.dma_start(Cb[:, :], bias_d[:, :])
            # weights staged through DVE so matmuls depend on one proc
            C = cpool.tile([128, 1536], f32r)
            nc.vector.tensor_copy(C[:, :], C0[:, :])
            W2 = C[:, 0:128]
            M8 = C[:, 128:256]
            M8n = C[:, 256:384]
            W1y1 = C[0:3, 384:512]
            W1y05 = C[0:3, 512:640]
            W1ut = C[0:5, 640:768]
            I128 = C[:, 784:912]
            PT2 = C[:, 912:1424]
            W3g = C[:, 768:771]
            W3m = C[:, 771:774]
            b1 = Cb[:, 0:1]
            b2 = Cb[:, 1:2]
            hb3 = Cb[0:3, 2:3]

            xy = bigpool.tile([3, SMAX * BC], f32r)
            xyf = bigpool.tile([3, SMAX * BC], f32)
            xs = bigpool.tile([5, SMAX * BC], f32r)
            xm = bigpool.tile([5, tw * BC], f32r)
            h2g = bigpool.tile([128, SMAX * BC], f32r)

            if skip:
                nc.gpsimd.memset(xy[:, :], 0.01)
                nc.gpsimd.memset(xyf[:, :], 0.01)
                nc.gpsimd.memset(xs[:, :], 0.01)
                nc.gpsimd.memset(xm[:, :], 0.01)

            prev_J = None
            for w, (i0, J) in enumerate(wins):
                S = J + 1
                # window statics (split + spread across DMA queues so the
                # refill chases the previous window's read wavefront)
                if "dma_in" not in skip:
                    for a, b in _pieces(S):
                        nc.gpsimd.dma_start(
                            xs[0:5, a * BC : b * BC].rearrange(
                                "p (s b) -> p s b", b=BC
                            ),
                            ugt_d[i0 + a : i0 + b].transpose([1, 0, 2]),
                        )
                    for a, b in _pieces(J):
                        nc.gpsimd.dma_start(
                            xm[0:5, a * BC : b * BC].rearrange(
                                "p (s b) -> p s b", b=BC
                            ),
                            umt_d[i0 + a : i0 + b].transpose([1, 0, 2]),
                        )
                # y_start
                ys = yspool.tile([3, BC], f32)
                if w == 0:
                    nc.gpsimd.dma_start(ys[:, :], y0_d[:, :])
                else:
                    nc.vector.tensor_copy(
                        ys[:, :], xyf[:, prev_J * BC : (prev_J + 1) * BC]
                    )

                gsc = _superchunks(S)
                msc = _superchunks(J)

                for m in range(ksweeps):
                    # interleaved emission: g0 g1 m0 g2 m1 ... (mid chunk k
                    # reads h2g/fg slices produced by grid chunks <= k+1)
                    order = []
                    for gi in range(len(gsc)):
                        order.append(("g", gsc[gi]))
                        if gi >= 1 and gi - 1 < len(msc):
                            order.append(("m", msc[gi - 1]))
                    for mi in range(len(gsc) - 1, len(msc)):
                        order.append(("m", msc[mi]))

                    for kind, (s0, ws) in order:
                        c0 = s0 * BC
                        wd = ws * BC
                        halves = [(h0, min(512, wd - h0)) for h0 in range(0, wd, 512)]
                        if kind == "g":
                            if m == 0 and "bcast" not in skip:
                                # init y rows for this chunk = y_start bcast
                                ysb = (
                                    ys[:, :]
                                    .rearrange("p (s b) -> p s b", s=1)
                                    .broadcast_to((3, ws, BC))
                                )
                                nc.vector.tensor_copy(
                                    xyf[:, c0 : c0 + wd].rearrange(
                                        "p (s b) -> p s b", b=BC
                                    ),
                                    ysb,
                                )
                                nc.vector.tensor_copy(
                                    xy[:, c0 : c0 + wd].rearrange(
                                        "p (s b) -> p s b", b=BC
                                    ),
                                    ysb,
                                )
                            psA = pspool.tile([128, wd], f32, tag="ps")
                            for h0, hw in halves:
                                a = c0 + h0
                                mm(psA[:, h0 : h0 + hw], W1y1,
                                   xy[:, a : a + hw], True, False)
                                mm(psA[:, h0 : h0 + hw], W1ut,
                                   xs[:, a : a + hw], False, False)
                                mm(psA[:, h0 : h0 + hw], I128,
                                   PT2[:, 0:hw], False, True)
                            h1 = h1pool.tile([128, wd], f32r, tag="h1")
                            nc.scalar.activation(h1[:, :], psA[:, :], Tanh, bias=b1)
                            for h0, hw in halves:
                                mm(psA[:, h0 : h0 + hw], W2,
                                   h1[:, h0 : h0 + hw], True, True)
                            nc.scalar.activation(
                                h2g[:, c0 : c0 + wd], psA[:, :], Tanh, bias=b2
                            )
                        else:
                            psM = pspool.tile([128, wd], f32, tag="ps")
                            for h0, hw in halves:
                                a = c0 + h0
                                mm(psM[:, h0 : h0 + hw], W1y05,
                                   xy[:, a : a + hw], True, False)
                                mm(psM[:, h0 : h0 + hw], W1y05,
                                   xy[:, a + BC : a + BC + hw], False, False)
                                mm(psM[:, h0 : h0 + hw], M8,
                                   h2g[:, a : a + hw], False, False)
                                mm(psM[:, h0 : h0 + hw], M8n,
                                   h2g[:, a + BC : a + BC + hw], False, False)
                                mm(psM[:, h0 : h0 + hw], W1ut,
                                   xm[:, a : a + hw], False, False)
                                mm(psM[:, h0 : h0 + hw], I128,
                                   PT2[:, 0:hw], False, True)
                            h1m = h1pool.tile([128, wd], f32r, tag="h1")
                            nc.scalar.activation(h1m[:, :], psM[:, :], Tanh, bias=b1)
                            for h0, hw in halves:
                                mm(psM[:, h0 : h0 + hw], W2,
                                   h1m[:, h0 : h0 + hw], True, True)
                            h2m = h2mpool.tile([128, wd], f32r, tag="h2m")
                            nc.scalar.activation(h2m[:, :], psM[:, :], Tanh, bias=b2)
                            # G accumulated fully in PSUM:
                            #   G = W3m'^T h2m_j + W3g'^T h2g_j + W3g'^T h2g_{j+1}
                            for h0, hw in halves:
                                a = c0 + h0
                                psG = psCpool.tile([3, 512], f32, tag="psC")
                                mm(psG[:, 0:hw], W3m, h2m[:, h0 : h0 + hw],
                                   True, False)
                                mm(psG[:, 0:hw], W3g, h2g[:, a : a + hw],
                                   False, False)
                                mm(psG[:, 0:hw], W3g,
                                   h2g[:, a + BC : a + BC + hw], False, True)
                                if gsbuf:
                                    Gs = gspool.tile([3, 512], f32, tag="gs")
                                    nc.vector.tensor_scalar_add(
                                        Gs[:, 0:hw], psG[:, 0:hw], hb3
                                    )
                                    gsrc = Gs
                                else:
                                    gsrc = psG
                                # serial scan in fp32: y_{j+1} = (G_j+h*b3)+y_j
                                nscan = hw // BC if "scan" not in skip else 0
                                for jj in range(nscan):
                                    j = s0 + h0 // BC + jj
                                    nc.vector.scalar_tensor_tensor(
                                        xyf[:, (j + 1) * BC : (j + 2) * BC],
                                        gsrc[:, jj * BC : (jj + 1) * BC],
                                        hb3,
                                        xyf[:, j * BC : (j + 1) * BC],
                                        ADD, ADD,
                                    )
                                if nscan:
                                    j0 = s0 + h0 // BC
                                    nc.vector.tensor_copy(
                                        xy[:, (j0 + 1) * BC : (j0 + 1 + nscan) * BC],
                                        xyf[:, (j0 + 1) * BC : (j0 + 1 + nscan) * BC],
                                    )
                # output (on the gpsimd queue, split so the WAR release of
                # the y rows chases the final scan wavefront)
                if "dma_out" not in skip:
                    for a, b in _pieces(J, 2):
                        nc.gpsimd.dma_start(
                            out_d[i0 + 1 + a : i0 + 1 + b].transpose([1, 0, 2]),
                            xyf[:, (1 + a) * BC : (1 + b) * BC].rearrange(
                                "p (s b) -> p s b", b=BC
                            ),
                        )
                prev_J = J
    nc.compile()
    return nc


def _prep_consts(W1, b1, W2, b2, W3, b3, h, pT=None):
    """Pack all weights into one (128, 1536) array (see _build_nc slices).
    pT: per-core (5, BC) parameters -> Pterm columns."""
    C = np.zeros((128, 1536), F32)
    C[:, 0:128] = W2
    M8 = (h / 8.0) * (W3 @ W1[0:3])          # (128,128)
    C[:, 128:256] = M8
    C[:, 256:384] = -M8
    C[0:3, 384:512] = W1[0:3]
    C[0:3, 512:640] = 0.5 * W1[0:3]
    C[0:5, 640:768] = np.concatenate([W1[3:7], W1[12:13]], axis=0)
    C[:, 784:912] = np.eye(128, dtype=F32)
    if pT is not None:
        pterm = (pT.T @ W1[7:12]).T.astype(F32)      # (128, BC)
        C[:, 912:1424] = np.tile(pterm, (1, 2))
    C[:, 768:771] = (h / 6.0) * W3
    C[:, 771:774] = (2.0 * h / 3.0) * W3
    Cb = np.zeros((128, 8), F32)
    Cb[:, 0] = b1
    Cb[:, 1] = b2
    Cb[0:3, 2] = h * b3
    return C, Cb


def _prep_core_inputs(c, y0, t, u, p, consts, T, tw):
    rows = slice(c * BC, (c + 1) * BC)
    W1_, b1_, W2_, b2_, W3_, b3_, h_ = consts
    u_c = np.ascontiguousarray(u[rows])          # (BC, T, 4)
    uT = np.transpose(u_c, (1, 2, 0))            # (T, 4, BC)
    ugt = np.empty((T, 5, BC), F32)
    ugt[:, 0:4] = uT
    ugt[:, 4] = t[:, None]
    umt = np.empty((T - 1, 5, BC), F32)
    umt[:, 0:4] = 0.5 * (uT[:-1] + uT[1:])
    tmid = 0.5 * (t[:-1] + t[1:])
    umt[:, 4] = tmid[:, None]
    pT = np.ascontiguousarray(p[rows].T)         # (5, BC)
    Cc, Cb = _prep_consts(W1_, b1_, W2_, b2_, W3_, b3_, h_, pT=pT)
    y0T = np.ascontiguousarray(y0[rows].T)       # (3, BC)
    return {
        "consts": Cc,
        "biases": Cb,
        "ugt": np.ascontiguousarray(ugt),
        "umt": np.ascontiguousarray(umt),
        "y0t": y0T,
    }


def run(inputs, T=T_FULL, tw=TW, ksweeps=K_SWEEPS, trace=False, cfg=None):
    from concourse.bass_utils import run_bass_kernel_spmd

    y0 = np.asarray(inputs["y0"], F32)
    t = np.asarray(inputs["t"], F32)
    u = np.asarray(inputs["u"], F32)
    p = np.asarray(inputs["p"], F32)
    W1 = np.asarray(inputs["W1"], F32)
    b1v = np.asarray(inputs["b1"], F32)
    W2 = np.asarray(inputs["W2"], F32)
    b2v = np.asarray(inputs["b2"], F32)
    W3 = np.asarray(inputs["W3"], F32)
    b3v = np.asarray(inputs["b3"], F32)
    h = float(t[1] - t[0])

    key = (T, tw, ksweeps, str(cfg))
    if key not in _CACHE:
        _CACHE[key] = _build_nc(T, tw, ksweeps, cfg=cfg)
    nc = _CACHE[key]

    consts = (W1, b1v, W2, b2v, W3, b3v, h)
    in_maps = [
        _prep_core_inputs(c, y0, t, u, p, consts, T, tw) for c in range(NCORES)
    ]
    res = run_bass_kernel_spmd(nc, in_maps, list(range(NCORES)), trace=trace)

    Bfull = y0.shape[0]
    out = np.empty((Bfull, T, 3), F32)
    for c in range(NCORES):
        out[c * BC : (c + 1) * BC] = res.results[c]["out"].transpose(2, 0, 1)
    out[:, 0, :] = y0
    return out, res


def kernel(**inputs):
    out, _ = run(inputs)
    return out



# revision 2
# speedup vs baseline: 1.3671x; 1.3671x over previous
"""Trainium2 Bass kernel for the NeuralBloch ODE problem — v2.

Scheme: windowed trapezoid collocation with a frozen vector field
(single sweep). Each window of J grid intervals evaluates the MLP at
all S=J+1 grid points with y frozen at the window-start value ys
(rel err ~3.8e-3 vs dopri5, tolerance 2e-2), then integrates
y_{j+1} = y_j + (h/2)(f_j + f_{j+1}).

The integration runs transposed: per-slice matmuls
Z^T[b, 3j:3j+3] = (h/2) h2_j^T W3 put batch on partitions, so the
window prefix-sum is 3 native tensor_tensor_scan instructions
(P'_j = sum Z_i + (j+1)h b3/2), and
y_{j+1} = P'_j + P'_{j+1} + (ys - Z_0 - h b3/2).

Each core owns 256 batch rows, split into 2 independent streams of 128
(interleaved chunk-wise) to fill stalls across the serial
window-to-window dependency. Window state round-trips through a tiny
DRAM line tensor: scan -> ytail [3,128] line-write -> stride-0
broadcast re-read into the next window's MLP input y-rows (both on the
ACT HWDGE ring; bulk u-loads and output dumps ride the SP HWDGE ring,
prefetched one window ahead).

Output leaves the device in scan-native layout outT[s, w, b, (j c)];
the host reassembles.
"""

import numpy as np

B_FULL = 2048
T_FULL = 2048
HID = 128
NCORES = 8
BC = B_FULL // NCORES   # 256 batch rows per core
NST = 2                 # streams per core
BCs = BC // NST         # 128 batch rows per stream
TW = 48                 # grid intervals per window
F32 = np.float32

_CACHE = {}


def _windows(T, tw):
    out = []
    i0 = 0
    while i0 < T - 1:
        J = min(tw, T - 1 - i0)
        out.append((i0, J))
        i0 += J
    return out


def _chunks(S, step=8):
    out = []
    a = 0
    while a < S:
        out.append((a, min(step, S - a)))
        a += step
    return out


def _halves(wd, step=512):
    return [(h0, min(step, wd - h0)) for h0 in range(0, wd, step)]


SKIP = ()
ACT_SPLIT = False


def _build_nc(T, tw):
    import concourse.bass as bass
    import concourse.bacc as bacc
    import concourse.mybir as mybir
    from concourse.tile import TileContext

    f32 = mybir.dt.float32
    f32r = mybir.dt.float32r
    Tanh = mybir.ActivationFunctionType.Tanh
    ADD = mybir.AluOpType.add
    SUB = mybir.AluOpType.subtract

    wins = _windows(T, tw)
    NW = len(wins)
    SMAX = tw + 1

    nc = bacc.Bacc(None)
    consts_d = nc.dram_tensor("consts", [128, 272], f32r, kind="ExternalInput")
    bias_d = nc.dram_tensor("biases", [128, 8], f32, kind="ExternalInput")
    ugt_d = nc.dram_tensor("ugt", [NST, 5, T * BCs], f32r,
                           kind="ExternalInput")
    y0t_d = nc.dram_tensor("y0t", [3, BC], f32r, kind="ExternalInput")
    y0tt_d = nc.dram_tensor("y0tt", [128, 3 * NST], f32, kind="ExternalInput")
    pt_d = nc.dram_tensor("pt", [5, BC], f32r, kind="ExternalInput")
    outT_d = nc.dram_tensor("outT", [NST, NW, 128, 3 * tw], f32,
                            kind="ExternalOutput")
    ytail_d = nc.dram_tensor("ytail", [NST, NW, 3, 128], f32,
                             kind="ExternalOutput")

    def u_load(xg, i0, S, s):
        nc.sync.dma_start(
            xg[3:8, :S * BCs],
            ugt_d[s, :, i0 * BCs:(i0 + S) * BCs],
        )

    with TileContext(nc) as tc:
        with (
            tc.tile_pool(name="const", bufs=1) as cpool,
            tc.tile_pool(name="big", bufs=1) as bigpool,
            tc.tile_pool(name="h1", bufs=(3 if tw <= 32 else 2)) as h1pool,
            tc.tile_pool(name="h2", bufs=(3 if tw <= 32 else 2)) as h2pool,
            tc.tile_pool(name="pp", bufs=3) as ppool,
            tc.tile_pool(name="ys", bufs=3) as yspool,
            tc.tile_pool(name="ps", bufs=3, space="PSUM") as pspool,
            tc.tile_pool(name="psg", bufs=2, space="PSUM") as psgpool,
        ):
            C = cpool.tile([128, 272], f32r)
            nc.sync.dma_start(C[:, :], consts_d[:, :])
            Cb = cpool.tile([128, 8], f32)
            nc.sync.dma_start(Cb[:, :], bias_d[:, :])
            y0tt = cpool.tile([128, 3 * NST], f32)
            nc.sync.dma_start(y0tt[:, :], y0tt_d[:, :])
            W2 = C[:, 0:128]
            W1f = C[0:13, 128:256]
            W3h = C[:, 256:259]
            b1 = Cb[:, 0:1]
            b2 = Cb[:, 1:2]
            chb3h = Cb[:, 2:5]   # h*b3/2, tiled over partitions

            XG = [[bigpool.tile([13, SMAX * BCs], f32r, name=f"xg{s}{par}")
                   for par in range(2)] for s in range(NST)]
            YT = [[bigpool.tile([128, 3 * tw], f32, name=f"yt{s}{par}")
                   for par in range(2)] for s in range(NST)]

            # p-rows: fill once per xg tile (DRAM stride-0 broadcast)
            for s in range(NST):
                for par in range(2):
                    nc.gpsimd.dma_start(
                        XG[s][par][8:13, :].rearrange("p (s b) -> p s b", b=BCs),
                        pt_d[:, s * BCs:(s + 1) * BCs]
                        .rearrange("p (s b) -> p s b", s=1)
                        .broadcast_to((5, SMAX, BCs)),
                    )

            # preamble: u-loads for windows 0 and 1, y bcast for window 0
            for w in range(min(2, NW)):
                i0, J = wins[w]
                for s in range(NST):
                    u_load(XG[s][w % 2], i0, J + 1, s)
            for s in range(NST):
                nc.sync.dma_start(
                    XG[s][0][0:3, :(wins[0][1] + 1) * BCs].rearrange(
                        "p (s b) -> p s b", b=BCs),
                    y0t_d[:, s * BCs:(s + 1) * BCs]
                    .rearrange("p (s b) -> p s b", s=1)
                    .broadcast_to((3, wins[0][1] + 1, BCs)),
                )

            for w, (i0, J) in enumerate(wins):
                S = J + 1
                par = w % 2
                chs = _chunks(S)

                # ---- stage-major emission: per stream, all W1+act1 then
                # all W2+act2 (ACT never waits on an in-chunk W2 round trip);
                # stream B's W1 block rides under stream A's act2 chain, and
                # Z blocks are cross-placed so they drain during the other
                # stream's activations ----
                psgt = [psgpool.tile([128, 3 * SMAX], f32, tag="psg",
                                     name=f"psgt{w}s{s}") for s in range(NST)]
                h2g = [h2pool.tile([128, S * BCs], f32r, tag="h2",
                                   name=f"h2g{w}s{s}") for s in range(NST)]
                h1g = [h1pool.tile([128, S * BCs], f32r, tag="h1",
                                   name=f"h1g{w}s{s}") for s in range(NST)]

                def stage1(s):
                    xg = XG[s][par]
                    for a, n in chs:
                        c0, wd = a * BCs, n * BCs
                        psA = pspool.tile([128, 1024], f32, tag="ps",
                                          name=f"psA{w}s{s}a{a}")
                        for h0, hw in _halves(wd):
                            nc.tensor.matmul(psA[:, h0:h0 + hw], W1f,
                                             xg[0:13, c0 + h0:c0 + h0 + hw],
                                             start=True, stop=True)
                        nc.scalar.activation(h1g[s][:, c0:c0 + wd],
                                             psA[:, 0:wd], Tanh, bias=b1)

                def stage2(s):
                    for a, n in chs:
                        c0, wd = a * BCs, n * BCs
                        psB = pspool.tile([128, 1024], f32, tag="ps",
                                          name=f"psB{w}s{s}a{a}")
                        for h0, hw in _halves(wd):
                            nc.tensor.matmul(psB[:, h0:h0 + hw], W2,
                                             h1g[s][:, c0 + h0:c0 + h0 + hw],
                                             start=True, stop=True)
                        nc.scalar.activation(h2g[s][:, c0:c0 + wd],
                                             psB[:, 0:wd], Tanh, bias=b2)

                def stageZ(s):
                    # plain fp32 (not fp32r): the fp32r PE mode requires an
                    # even moving dim, and this out is [128, 3]
                    if "z" in SKIP:
                        return
                    for j in range(S):
                        nc.tensor.matmul(
                            psgt[s][:, 3 * j:3 * j + 3],
                            h2g[s][:, j * BCs:(j + 1) * BCs].bitcast(f32),
                            W3h.bitcast(f32), start=True, stop=True)

                def tail(s):
                    yT = YT[s][par]
                    P = ppool.tile([128, 3 * SMAX], f32, tag="pp",
                                   name=f"P{w}s{s}")
                    ysm = yspool.tile([128, 3], f32, tag="ys",
                                      name=f"ysm{w}s{s}")
                    if "z" in SKIP:
                        nc.gpsimd.memset(yT[:, 0:3 * J], 0.01)
                    else:
                        # P'_j = sum_{i<=j} Z_i + (j+1) h b3/2
                        for c in range(3):
                            nc.vector.tensor_tensor_scan(
                                P[:, c:3 * S:3],
                                psgt[s][:, c:3 * S:3],
                                chb3h[:, c:c + 1].broadcast_to((128, S)),
                                0.0, ADD, ADD,
                            )
                        # ysm = ys - Z_0 - h b3/2
                        nc.vector.tensor_tensor(ysm[:, :], psgt[s][:, 0:3],
                                                chb3h[:, :], ADD)
                        if w == 0:
                            ysT = y0tt[:, 3 * s:3 * s + 3]
                        else:
                            Jp = wins[w - 1][1]
                            ysT = YT[s][1 - par][:, 3 * (Jp - 1):3 * Jp]
                        nc.vector.tensor_tensor(ysm[:, :], ysT, ysm[:, :], SUB)
                        # y_{j+1} = P'_j + P'_{j+1} + ysm
                        nc.vector.tensor_tensor(yT[:, 0:3 * J], P[:, 0:3 * J],
                                                P[:, 3:3 * S], ADD)
                        nc.vector.tensor_tensor(
                            yT[:, 0:3 * J].rearrange("p (j c) -> p j c", c=3),
                            yT[:, 0:3 * J].rearrange("p (j c) -> p j c", c=3),
                            ysm[:, :].rearrange("p (j c) -> p j c", j=1)
                            .broadcast_to((128, J, 3)),
                            ADD)
                    if w + 1 < NW:
                        nc.sync.dma_start(
                            ytail_d[s, w].transpose([1, 0]),
                            yT[:, 3 * (J - 1):3 * J],
                        )
                        # broadcast re-read feeds window w+1's y rows
                        Sn = wins[w + 1][1] + 1
                        nc.sync.dma_start(
                            XG[s][1 - par][0:3, :Sn * BCs].rearrange(
                                "p (s b) -> p s b", b=BCs),
                            ytail_d[s, w].bitcast(f32r)
                            .rearrange("p (s b) -> p s b", s=1)
                            .broadcast_to((3, Sn, BCs)),
                        )
                    # output dump on the otherwise-idle Pool ring
                    nc.gpsimd.dma_start(outT_d[s, w, :, 0:3 * J], yT[:, 0:3 * J])

                stage1(0)
                stage2(0)
                stage1(1)
                stageZ(0)
                stage2(1)
                tail(0)
                stageZ(1)
                tail(1)

                # prefetch u for window w+2 into this window's parity tile
                # (emitted after w's reads so WAR ordering is correct; its dep
                # clears mid-window w, after this window's bcasts on SP)
                if w + 2 < NW:
                    i0n, Jn = wins[w + 2]
                    for s in range(NST):
                        u_load(XG[s][par], i0n, Jn + 1, s)
    nc.compile()
    return nc


def _prep_core_inputs(c, y0, t, u, p, W1, b1v, W2, b2v, W3, b3v, h, T):
    rows = slice(c * BC, (c + 1) * BC)
    u_c = np.ascontiguousarray(u[rows])            # (BC, T, 4)
    ugt = np.empty((NST, 5, T, BCs), F32)
    for s in range(NST):
        us_ = u_c[s * BCs:(s + 1) * BCs]           # (BCs, T, 4)
        ugt[s, 0:4] = np.transpose(us_, (2, 1, 0))
        ugt[s, 4] = t[:, None]
    ugt = ugt.reshape(NST, 5, T * BCs)

    C = np.zeros((128, 272), F32)
    C[:, 0:128] = W2
    # W1full rows: y(3), u(4), t(1), p(5) — matches xg partition rows
    C[0:3, 128:256] = W1[0:3]
    C[3:7, 128:256] = W1[3:7]
    C[7, 128:256] = W1[12]
    C[8:13, 128:256] = W1[7:12]
    C[:, 256:259] = (h / 2.0) * W3
    Cb = np.zeros((128, 8), F32)
    Cb[:, 0] = b1v
    Cb[:, 1] = b2v
    Cb[:, 2:5] = (h / 2.0) * b3v[None, :]

    y0c = y0[rows]                                 # (BC, 3)
    y0t = np.ascontiguousarray(y0c.T)              # (3, BC)
    y0tt = np.empty((128, 3 * NST), F32)
    for s in range(NST):
        y0tt[:, 3 * s:3 * s + 3] = y0c[s * BCs:(s + 1) * BCs]
    pt = np.ascontiguousarray(p[rows].T)           # (5, BC)
    return {
        "consts": C,
        "biases": Cb,
        "ugt": np.ascontiguousarray(ugt),
        "y0t": y0t,
        "y0tt": y0tt,
        "pt": pt,
    }


def run(inputs, T=T_FULL, tw=None, trace=False):
    if tw is None:
        tw = TW
    from concourse.bass_utils import run_bass_kernel_spmd

    y0 = np.asarray(inputs["y0"], F32)
    t = np.asarray(inputs["t"], F32)
    u = np.asarray(inputs["u"], F32)
    p = np.asarray(inputs["p"], F32)
    W1 = np.asarray(inputs["W1"], F32)
    b1v = np.asarray(inputs["b1"], F32)
    W2 = np.asarray(inputs["W2"], F32)
    b2v = np.asarray(inputs["b2"], F32)
    W3 = np.asarray(inputs["W3"], F32)
    b3v = np.asarray(inputs["b3"], F32)
    h = float(t[1] - t[0])

    key = (T, tw)
    if key not in _CACHE:
        _CACHE[key] = _build_nc(T, tw)
    nc = _CACHE[key]

    in_maps = [
        _prep_core_inputs(c, y0, t, u, p, W1, b1v, W2, b2v, W3, b3v, h, T)
        for c in range(NCORES)
    ]
    res = run_bass_kernel_spmd(nc, in_maps, list(range(NCORES)), trace=trace)

    wins = _windows(T, tw)
    Bfull = y0.shape[0]
    out = np.empty((Bfull, T, 3), F32)
    for c in range(NCORES):
        outT = res.results[c]["outT"]              # (NST, NW, 128, 3*tw)
        for s in range(NST):
            r0 = c * BC + s * BCs
            for w, (i0, J) in enumerate(wins):
                out[r0:r0 + BCs, i0 + 1:i0 + 1 + J] = (
                    outT[s, w, :, :3 * J].reshape(BCs, J, 3))
    out[:, 0, :] = y0
    return out, res


def kernel(**inputs):
    out, _ = run(inputs)
    return out
